# Initial kernel scaffold
#
"""Trainium2 Bass kernel for nn_MoDBlock (mixture-of-depths block), 8 cores.

Contract: kernel(**inputs) takes FULL inputs (x (4,4096,2048) f32,
position_ids (4,4096) i32 [arange per spec], router_w, norm weights, qkv_w,
out_w, w1/w2/w3) and returns the FULL (4,4096,2048) f32 output.

Sharding: 4 pairs x 2 cores; pair g owns batch row b=g. Both cores of a pair
run the router (fp32 scores + tie), exact top-512 via gpsimd kth_largest ->
threshold -> sparse_gather compaction (ascending token order, matching
jax.lax.top_k + sort semantics incl. stable tie handling), and dma_gather of
the selected rows. Core half h processes selected ranks [256h, 256h+256):
q/attention-out/out-proj/SwiGLU for its ranks; k/v projections for all 512.
Causal mask on ranks == mask on original positions (positions ascending).
Block math in bf16 with fp32 accumulation; router and residuals in fp32.
Host only shards inputs, converts weights to bf16, and scatters per-core
outputs into a copy of x (out[b, idx[...]] = proc).
"""


import os
import numpy as np
import ml_dtypes
import concourse.bass as bass
import concourse.bacc as bacc
import concourse.mybir as mybir
import concourse.tile as tile
from concourse import library_config
from concourse.tile_rust import add_dep_helper

F32 = mybir.dt.float32
BF16 = mybir.dt.bfloat16
AF = mybir.ActivationFunctionType
OP = mybir.AluOpType

B, T, D, H = 4, 4096, 2048, 16
HD = 128
K = 512
KC = 256          # tokens per core
DFF = 5461
DFFP = 5504       # padded to 43*128
NFC = DFFP // 128  # 43
EPS = 1e-6
ISQ = 1.0 / np.sqrt(128.0)
QUANT = 1.0 - 510.5 / 4095.0  # k_adj = 510 -> out {lerp, desc[511]}
DEBUG = False
PHASE_LIMIT = 8


def build_kernel(tc: tile.TileContext, outs, ins):
    nc = tc.nc
    xb = ins["xb"]
    proc_o, idx_o, nf_o = outs["proc"], outs["idxo"], outs["nfo"]

    _open = {}

    def popen(name, side="left", **kw):
        cm = tc.tile_pool(name=name, side=side, **kw)
        _open[name] = cm
        return cm.__enter__()

    def pclose(name):
        _open.pop(name).__exit__(None, None, None)

    const = popen("const", bufs=1)
    small = popen("small", bufs=1)
    x1p = popen("x1_pool", bufs=1)          # lives A->H
    xown_pool = popen("xown_pool", bufs=1)  # lives A->E
    h1T_pool = popen("h1T_pool", side="right", bufs=1)  # lives A->C

    x1_t = x1p.tile([128, 2, 2048], F32, name="x1_t")
    xown_t = xown_pool.tile([128, 2, 2048], F32, name="xown_t")
    h1T = h1T_pool.tile([128, 16, 512], BF16, name="h1T")
    h1sel = h1T_pool.tile([128, 16, 256], BF16, name="h1sel")

    # ---- constants ----
    rw_t = const.tile([128, 2048], F32)
    nc.sync.dma_start(rw_t[:], ins["rw"][:])
    tie_t = const.tile([128, 32], F32)
    nc.sync.dma_start(tie_t[:], ins["tie"][:])
    iota_t = const.tile([128, 32], F32)
    nc.sync.dma_start(iota_t[:], ins["iota1"][:])
    ones1_t = const.tile([1, 128], F32)
    nc.sync.dma_start(ones1_t[:], ins["ones1"][:])
    ident_t = const.tile([128, 128], BF16)
    nc.sync.dma_start(ident_t[:], ins["identb"][:])
    n1w_t = const.tile([128, 2048], BF16)
    nc.sync.dma_start(n1w_t[:], ins["n1w"][:])
    n2w_t = const.tile([128, 2048], BF16)
    nc.sync.dma_start(n2w_t[:], ins["n2w"][:])
    cmask_t = const.tile([128, 4, 256], BF16)
    nc.sync.dma_start(cmask_t[:], ins["cmask"][:])
    qs0_t = const.tile([128, 1], F32)
    nc.sync.dma_start(qs0_t[:], ins["qs0"][:])
    qs1_t = const.tile([128, 1], F32)
    nc.sync.dma_start(qs1_t[:], ins["qs1"][:])
    onesk_t = const.tile([128, 1], BF16)
    nc.vector.memset(onesk_t[:], 1.0)
    eps_t = const.tile([128, 1], F32)
    nc.vector.memset(eps_t[:], EPS)

    # =========== Phase A: router scores + topk + gather ===========
    S_t = small.tile([128, 32], F32)
    with tc.tile_pool(name="xstream", side="right", bufs=4) as xs:
        for k in range(32):
            xk = xs.tile([128, 2048], F32, tag="xk", name=f"xk{k}")
            nc.sync.dma_start(xk[:], xb[k * 128:(k + 1) * 128, :])
            nc.vector.scalar_tensor_tensor(
                out=xk[:], in0=xk[:], scalar=1.0, in1=rw_t[:],
                op0=OP.mult, op1=OP.mult, accum_out=S_t[:, k:k + 1],
            )
    nc.vector.tensor_add(out=S_t[:], in0=S_t[:], in1=tie_t[:])

    kth_t = small.tile([1, 2], F32)
    lib_attn = nc.gpsimd.load_library(library_config.attn)
    kth = nc.gpsimd.kth_largest(
        kth_t[:], S_t[:], n_per_lane=32, k=510, quantile=QUANT)
    add_dep_helper(kth.ins, lib_attn.ins, reason="lib attn first")

    th_t = small.tile([128, 1], F32)
    with tc.tile_pool(name="psA", bufs=1, space="PSUM") as psA:
        th_ps = psA.tile([128, 1], F32)
        nc.tensor.matmul(th_ps[:], ones1_t[:], kth_t[:, 1:2],
                         start=True, stop=True)
        nc.vector.tensor_copy(th_t[:], th_ps[:])

    cand_t = small.tile([128, 32], F32)
    nc.vector.scalar_tensor_tensor(
        out=cand_t[:], in0=S_t[:], scalar=th_t[:], in1=iota_t[:],
        op0=OP.is_ge, op1=OP.mult)
    nc.vector.tensor_scalar_add(cand_t[:], cand_t[:], -1.0)

    c16_t = small.tile([16, 32, 8], F32)
    for pi in range(8):
        nc.sync.dma_start(c16_t[:, :, pi], cand_t[pi * 16:(pi + 1) * 16, :])

    sg_t = small.tile([16, 33], F32)
    nf_t = small.tile([1, 1], mybir.dt.uint32)
    lib_sg = nc.gpsimd.load_library(library_config.sparse_gather)
    sg = nc.gpsimd.sparse_gather(
        sg_t[:], c16_t[:].rearrange("p k j -> p (k j)"), num_found=nf_t[:])
    add_dep_helper(lib_sg.ins, kth.ins, reason="lib switch after kth")
    add_dep_helper(sg.ins, lib_sg.ins, reason="sg after lib")
    nc.sync.dma_start(nf_o[:], nf_t[:])

    idx32_t = small.tile([16, 32], mybir.dt.int32)
    nc.vector.tensor_copy(idx32_t[:], sg_t[:, 0:32])
    nc.sync.dma_start(idx_o.rearrange("(f p) -> p f", p=16), idx32_t[:])

    idx16_t = small.tile([16, 32], mybir.dt.int16)
    nc.vector.tensor_copy(idx16_t[:], sg_t[:, 0:32])
    idx128_t = small.tile([128, 32], mybir.dt.int16)
    for g in range(8):
        nc.sync.dma_start(idx128_t[g * 16:(g + 1) * 16, :], idx16_t[:])

    x_sel = popen("x_sel_pool", bufs=1)
    xsel_t = x_sel.tile([128, 4, 2048], F32, name="xsel_t")
    lib_mlp = nc.gpsimd.load_library(library_config.mlp)
    gat = nc.gpsimd.dma_gather(
        xsel_t[:], xb[:], idx128_t[:], K, K, 2048)
    add_dep_helper(lib_mlp.ins, sg.ins, reason="lib switch after sg")
    add_dep_helper(gat.ins, lib_mlp.ins, reason="gather after lib")


    if PHASE_LIMIT <= 1:
        nc.sync.dma_start(proc_o[0:128, :], x1_t[:, 0, :] if 1 >= 5 else xown_t[:, 0, :])
        for nm in list(_open)[::-1]:
            pclose(nm)
        return
    # =========== Phase B: norm1, h1, h1T, blends ===========
    rs1_t = small.tile([128, 4], F32)
    sq1_t = small.tile([128, 4], F32)
    with tc.tile_pool(name="scratch", bufs=2) as scr:
        for c in range(4):
            sc = scr.tile([128, 2048], F32, tag="sc", name=f"sc{c}")
            nc.vector.scalar_tensor_tensor(
                out=sc[:], in0=xsel_t[:, c, :], scalar=1.0,
                in1=xsel_t[:, c, :], op0=OP.mult, op1=OP.mult,
                accum_out=sq1_t[:, c:c + 1])
    nc.scalar.activation(rs1_t[:], sq1_t[:], AF.Sqrt,
                         scale=1.0 / 2048.0, bias=eps_t[:])
    nc.vector.reciprocal(rs1_t[:], rs1_t[:])

    with (
        tc.tile_pool(name="h1_pool", bufs=1) as h1p,
        tc.tile_pool(name="psT", bufs=4, space="PSUM") as psT,
    ):
        h1_t = h1p.tile([128, 4, 2048], BF16, name="h1_t")
        for c in range(4):
            nc.vector.scalar_tensor_tensor(
                out=h1_t[:, c, :], in0=xsel_t[:, c, :],
                scalar=rs1_t[:, c:c + 1], in1=n1w_t[:],
                op0=OP.mult, op1=OP.mult)
        for c in range(4):
            for dc in range(16):
                pt = psT.tile([128, 128], BF16, tag="pt", name=f"pt{c}_{dc}")
                nc.tensor.transpose(
                    pt[:], h1_t[:, c, dc * 128:(dc + 1) * 128], ident_t[:])
                nc.scalar.activation(
                    h1T[:, dc, c * 128:(c + 1) * 128], pt[:], AF.Copy)

    # x_own / h1sel blends
    for qt in range(2):
        nc.vector.tensor_scalar_mul(
            xown_t[:, qt, :], xsel_t[:, qt, :], qs0_t[:])
        nc.vector.scalar_tensor_tensor(
            out=xown_t[:, qt, :], in0=xsel_t[:, 2 + qt, :],
            scalar=qs1_t[:], in1=xown_t[:, qt, :],
            op0=OP.mult, op1=OP.add)
    for dc in range(16):
        nc.vector.tensor_scalar_mul(
            h1sel[:, dc, :], h1T[:, dc, 0:256], qs0_t[:])
        nc.vector.scalar_tensor_tensor(
            out=h1sel[:, dc, :], in0=h1T[:, dc, 256:512],
            scalar=qs1_t[:], in1=h1sel[:, dc, :],
            op0=OP.mult, op1=OP.add)
    pclose("x_sel_pool")


    if PHASE_LIMIT <= 2:
        nc.sync.dma_start(proc_o[0:128, :], x1_t[:, 0, :] if 2 >= 5 else xown_t[:, 0, :])
        for nm in list(_open)[::-1]:
            pclose(nm)
        return
    # =========== Phase C: qkv projections ===========
    qkvp = popen("qkv_pool", bufs=1)
    qT = qkvp.tile([128, 16, 256], BF16, name="qT")
    kT = qkvp.tile([128, 16, 512], BF16, name="kT")
    V = qkvp.tile([128, 4, 2048], BF16, name="V")

    with (
        tc.tile_pool(name="wqk_stream", bufs=3) as wqs,
        tc.tile_pool(name="psC", bufs=2, space="PSUM") as psC,
    ):
        for jc in range(16):
            wqc = wqs.tile([128, 16, 128], BF16, tag="wqc", name=f"wq{jc}")
            nc.sync.dma_start(
                wqc[:], ins["wq"].rearrange("(dc p) j -> p dc j", p=128)
                [:, :, jc * 128:(jc + 1) * 128])
            pq = psC.tile([128, 256], F32, tag="pq", bufs=2, name=f"pq{jc}")
            for dc in range(16):
                nc.tensor.matmul(pq[:], wqc[:, dc, :], h1sel[:, dc, :],
                                 start=(dc == 0), stop=(dc == 15))
            nc.scalar.activation(qT[:, jc, :], pq[:], AF.Copy)
        for jc in range(16):
            wkc = wqs.tile([128, 16, 128], BF16, tag="wqc", name=f"wk{jc}")
            nc.sync.dma_start(
                wkc[:], ins["wk"].rearrange("(dc p) j -> p dc j", p=128)
                [:, :, jc * 128:(jc + 1) * 128])
            pk = psC.tile([128, 512], F32, tag="pk", bufs=2, name=f"pk{jc}")
            for dc in range(16):
                nc.tensor.matmul(pk[:], wkc[:, dc, :], h1T[:, dc, :],
                                 start=(dc == 0), stop=(dc == 15))
            nc.scalar.activation(kT[:, jc, :], pk[:], AF.Copy)
        for vc in range(4):
            pvs = [psC.tile([128, 512], F32, tag="pv", bufs=4,
                            name=f"pv{vc}_{i}") for i in range(4)]
            for dc in range(16):
                wvc = wqs.tile([128, 512], BF16, tag="wvc", name=f"wv{vc}_{dc}")
                nc.sync.dma_start(
                    wvc[:], ins["wv"][dc * 128:(dc + 1) * 128,
                                      vc * 512:(vc + 1) * 512])
                for tc4 in range(4):
                    nc.tensor.matmul(
                        pvs[tc4][:], h1T[:, dc, tc4 * 128:(tc4 + 1) * 128],
                        wvc[:], start=(dc == 0), stop=(dc == 15))
            for tc4 in range(4):
                nc.scalar.activation(
                    V[:, tc4, vc * 512:(vc + 1) * 512], pvs[tc4][:], AF.Copy)
    pclose("h1T_pool")


    if PHASE_LIMIT <= 3:
        nc.sync.dma_start(proc_o[0:128, :], x1_t[:, 0, :] if 3 >= 5 else xown_t[:, 0, :])
        for nm in list(_open)[::-1]:
            pclose(nm)
        return
    # =========== Phase D: attention ===========
    attp = popen("att_pool", side="right", bufs=1)
    o_t = attp.tile([128, 2, 16, 128], BF16, name="o_t")
    oT = attp.tile([128, 16, 256], BF16, name="oT")
    with (
        tc.tile_pool(name="pT_pool", bufs=2) as pTp,
        tc.tile_pool(name="lrow_pool", bufs=2) as lrp,
        tc.tile_pool(name="psD", bufs=2, space="PSUM") as psD,
        tc.tile_pool(name="psL", bufs=2, space="PSUM") as psL,
    ):
        for h in range(16):
            pT = pTp.tile([128, 4, 256], BF16, tag="pT", name=f"pT{h}")
            for kc in range(4):
                ss = psD.tile([128, 256], F32, tag="ss", name=f"ss{h}_{kc}")
                nc.tensor.matmul(
                    ss[:], kT[:, h, kc * 128:(kc + 1) * 128], qT[:, h, :],
                    start=True, stop=True)
                pe_t = pTp.tile([128, 256], F32, tag="pe", name=f"pe{h}_{kc}")
                nc.scalar.activation(pe_t[:], ss[:], AF.Exp, scale=ISQ)
                nc.vector.tensor_mul(
                    out=pT[:, kc, :], in0=pe_t[:], in1=cmask_t[:, kc, :])
            lps = psL.tile([1, 256], F32, tag="lps", name=f"lps{h}")
            for kc in range(4):
                nc.tensor.matmul(lps[:], onesk_t[:], pT[:, kc, :],
                                 start=(kc == 0), stop=(kc == 3))
            lrow = lrp.tile([1, 256], F32, tag="lrow", name=f"lrow{h}")
            nc.scalar.activation(lrow[:], lps[:], AF.Copy)
            lcol = lrp.tile([128, 2], F32, tag="lcol", name=f"lcol{h}")
            for qt in range(2):
                nc.sync.dma_start(
                    lcol[:, qt:qt + 1],
                    lrow[0:1, qt * 128:(qt + 1) * 128])
            rL = lrp.tile([128, 2], F32, tag="rL", name=f"rL{h}")
            nc.vector.reciprocal(rL[:], lcol[:])
            if DEBUG and h == 0:
                nc.sync.dma_start(outs["dbg_pT"][:], pT[:])
                nc.sync.dma_start(outs["dbg_rL"][:], rL[:])
                nc.sync.dma_start(outs["dbg_lrow"][:], lrow[:])
            for qt in range(2):
                po = psD.tile([128, 128], F32, tag="po", name=f"po{h}_{qt}")
                for kc in range(4):
                    nc.tensor.matmul(
                        po[:], pT[:, kc, qt * 128:(qt + 1) * 128],
                        V[:, kc, h * 128:(h + 1) * 128],
                        start=(kc == 0), stop=(kc == 3))
                nc.scalar.activation(o_t[:, qt, h, :], po[:], AF.Copy,
                                     scale=rL[:, qt:qt + 1])
    with tc.tile_pool(name="psT2", bufs=2, space="PSUM") as psT2:
        for qt in range(2):
            for h in range(16):
                pt = psT2.tile([128, 128], BF16, tag="pt2",
                               name=f"pt2_{qt}_{h}")
                nc.tensor.transpose(pt[:], o_t[:, qt, h, :], ident_t[:])
                nc.scalar.activation(
                    oT[:, h, qt * 128:(qt + 1) * 128], pt[:], AF.Copy)
    if DEBUG:
        nc.sync.dma_start(outs["dbg_qT"][:], qT[:])
        nc.sync.dma_start(outs["dbg_kT"][:], kT[:])
        nc.sync.dma_start(outs["dbg_V"][:], V[:])
        nc.sync.dma_start(outs["dbg_o"][:], o_t[:])
    pclose("qkv_pool")


    if PHASE_LIMIT <= 4:
        nc.sync.dma_start(proc_o[0:128, :], x1_t[:, 0, :] if 4 >= 5 else xown_t[:, 0, :])
        for nm in list(_open)[::-1]:
            pclose(nm)
        return
    # =========== Phase E: out-proj + residual -> x1 ===========
    with (
        tc.tile_pool(name="ow_stream", bufs=2) as ows,
        tc.tile_pool(name="psE", bufs=2, space="PSUM") as psE,
    ):
        for nc4 in range(4):
            owc = ows.tile([128, 16, 512], BF16, tag="owc", name=f"ow{nc4}")
            nc.sync.dma_start(
                owc[:], ins["ow"].rearrange("(oc p) n -> p oc n", p=128)
                [:, :, nc4 * 512:(nc4 + 1) * 512])
            for qt in range(2):
                poo = psE.tile([128, 512], F32, tag="poo",
                               name=f"poo{nc4}_{qt}")
                for oc in range(16):
                    nc.tensor.matmul(
                        poo[:], oT[:, oc, qt * 128:(qt + 1) * 128],
                        owc[:, oc, :], start=(oc == 0), stop=(oc == 15))
                nc.vector.tensor_add(
                    out=x1_t[:, qt, nc4 * 512:(nc4 + 1) * 512],
                    in0=poo[:], in1=xown_t[:, qt, nc4 * 512:(nc4 + 1) * 512])
    if DEBUG:
        nc.sync.dma_start(outs["dbg_x1"][:], x1_t[:])
        nc.sync.dma_start(outs["dbg_xown"][:], xown_t[:])
    pclose("att_pool")
    pclose("xown_pool")


    if PHASE_LIMIT <= 5:
        nc.sync.dma_start(proc_o[0:128, :], x1_t[:, 0, :] if 5 >= 5 else xown_t[:, 0, :])
        for nm in list(_open)[::-1]:
            pclose(nm)
        return
    # =========== Phase F: norm2 + h2T ===========
    rs2_t = small.tile([128, 2], F32)
    sq2_t = small.tile([128, 2], F32)
    zzp = popen("zz_pool", side="right", bufs=1)
    zz = zzp.tile([128, NFC, 256], BF16, name="zz")
    h2Tp = popen("h2T_pool", side="right", bufs=1)
    h2T = h2Tp.tile([128, 16, 256], BF16, name="h2T")

    with tc.tile_pool(name="scratch2", bufs=2) as scr2:
        for c in range(2):
            sc = scr2.tile([128, 2048], F32, tag="sc2", name=f"sc2_{c}")
            nc.vector.scalar_tensor_tensor(
                out=sc[:], in0=x1_t[:, c, :], scalar=1.0,
                in1=x1_t[:, c, :], op0=OP.mult, op1=OP.mult,
                accum_out=sq2_t[:, c:c + 1])
    nc.scalar.activation(rs2_t[:], sq2_t[:], AF.Sqrt,
                         scale=1.0 / 2048.0, bias=eps_t[:])
    nc.vector.reciprocal(rs2_t[:], rs2_t[:])

    with (
        tc.tile_pool(name="h2_pool", bufs=1) as h2p,
        tc.tile_pool(name="psT3", bufs=2, space="PSUM") as psT3,
    ):
        h2_t = h2p.tile([128, 2, 2048], BF16, name="h2_t")
        for c in range(2):
            nc.vector.scalar_tensor_tensor(
                out=h2_t[:, c, :], in0=x1_t[:, c, :],
                scalar=rs2_t[:, c:c + 1], in1=n2w_t[:],
                op0=OP.mult, op1=OP.mult)
        for c in range(2):
            for dc in range(16):
                pt = psT3.tile([128, 128], BF16, tag="pt3",
                               name=f"pt3_{c}_{dc}")
                nc.tensor.transpose(
                    pt[:], h2_t[:, c, dc * 128:(dc + 1) * 128], ident_t[:])
                nc.scalar.activation(
                    h2T[:, dc, c * 128:(c + 1) * 128], pt[:], AF.Copy)


    if PHASE_LIMIT <= 6:
        nc.sync.dma_start(proc_o[0:128, :], x1_t[:, 0, :] if 6 >= 5 else xown_t[:, 0, :])
        for nm in list(_open)[::-1]:
            pclose(nm)
        return
    # =========== Phase G: FFN ===========
    with (
        tc.tile_pool(name="w12_stream", bufs=3) as w12s,
        tc.tile_pool(name="sig_pool", bufs=3) as sigp,
        tc.tile_pool(name="psG", bufs=2, space="PSUM") as psG,
    ):
        for fc in range(NFC):
            w1c = w12s.tile([128, 16, 128], BF16, tag="w1c", name=f"w1c{fc}")
            nc.sync.dma_start(
                w1c[:], ins["w1"].rearrange("(dc p) f -> p dc f", p=128)
                [:, :, fc * 128:(fc + 1) * 128])
            w2c = w12s.tile([128, 16, 128], BF16, tag="w2c", name=f"w2c{fc}")
            nc.sync.dma_start(
                w2c[:], ins["w2"].rearrange("(dc p) f -> p dc f", p=128)
                [:, :, fc * 128:(fc + 1) * 128])
            p1 = psG.tile([128, 256], F32, tag="p1", name=f"p1_{fc}")
            p2 = psG.tile([128, 256], F32, tag="p2", name=f"p2_{fc}")
            for dc in range(16):
                nc.tensor.matmul(p1[:], w1c[:, dc, :], h2T[:, dc, :],
                                 start=(dc == 0), stop=(dc == 15))
            for dc in range(16):
                nc.tensor.matmul(p2[:], w2c[:, dc, :], h2T[:, dc, :],
                                 start=(dc == 0), stop=(dc == 15))
            sg2 = sigp.tile([128, 256], BF16, tag="sg2", name=f"sg2_{fc}")
            nc.scalar.activation(sg2[:], p1[:], AF.Sigmoid)
            s1 = sigp.tile([128, 256], BF16, tag="s1", name=f"s1_{fc}")
            nc.vector.tensor_mul(out=s1[:], in0=sg2[:], in1=p1[:])
            nc.vector.tensor_mul(out=zz[:, fc, :], in0=s1[:], in1=p2[:])
    pclose("h2T_pool")


    if PHASE_LIMIT <= 7:
        nc.sync.dma_start(proc_o[0:128, :], x1_t[:, 0, :] if 7 >= 5 else xown_t[:, 0, :])
        for nm in list(_open)[::-1]:
            pclose(nm)
        return
    # =========== Phase H: w3 + residual -> proc ===========
    procp = popen("proc_pool", bufs=1)
    proc_t = procp.tile([128, 2, 2048], F32, name="proc_t")
    with (
        tc.tile_pool(name="w3_stream", bufs=4) as w3s,
        tc.tile_pool(name="psH", bufs=1, space="PSUM") as psH,
    ):
        pffs = {}
        for qt in range(2):
            for nc4 in range(4):
                pffs[(qt, nc4)] = psH.tile(
                    [128, 512], F32, tag=f"pff{qt}{nc4}",
                    name=f"pff{qt}{nc4}")
        for fc in range(NFC):
            w3c = w3s.tile([128, 2048], BF16, tag="w3c", name=f"w3c{fc}")
            nc.sync.dma_start(w3c[:], ins["w3"][fc * 128:(fc + 1) * 128, :])
            for qt in range(2):
                for nc4 in range(4):
                    nc.tensor.matmul(
                        pffs[(qt, nc4)][:],
                        zz[:, fc, qt * 128:(qt + 1) * 128],
                        w3c[:, nc4 * 512:(nc4 + 1) * 512],
                        start=(fc == 0), stop=(fc == NFC - 1))
        for qt in range(2):
            for nc4 in range(4):
                nc.vector.tensor_add(
                    out=proc_t[:, qt, nc4 * 512:(nc4 + 1) * 512],
                    in0=pffs[(qt, nc4)][:],
                    in1=x1_t[:, qt, nc4 * 512:(nc4 + 1) * 512])
    for qt in range(2):
        nc.sync.dma_start(proc_o[qt * 128:(qt + 1) * 128, :],
                          proc_t[:, qt, :])
    pclose("proc_pool")
    pclose("zz_pool")
    pclose("x1_pool")
    pclose("xown_pool") if "xown_pool" in _open else None
    pclose("small")
    pclose("const")


# ======================= host side =======================

def host_constants(inputs):
    """Shared per-core constants from full inputs (numpy)."""
    f32 = np.float32
    bf = ml_dtypes.bfloat16
    qkv_w = np.asarray(inputs["qkv_w"], f32)
    con = {}
    con["rw"] = np.broadcast_to(
        np.asarray(inputs["router_w"], f32)[None, :], (128, 2048)).copy()
    tie = (np.arange(T, dtype=f32) * np.float32(1e-6))
    con["tie"] = tie.reshape(32, 128).T.copy()
    con["iota1"] = (np.arange(T, dtype=f32) + 1.0).reshape(32, 128).T.copy().astype(f32)
    con["ones1"] = np.ones((1, 128), f32)
    con["identb"] = np.eye(128, dtype=f32).astype(bf)
    con["n1w"] = np.broadcast_to(
        np.asarray(inputs["norm1_w"], f32)[None, :], (128, 2048)).astype(bf)
    con["n2w"] = np.broadcast_to(
        np.asarray(inputs["norm2_w"], f32)[None, :], (128, 2048)).astype(bf)
    con["wq"] = qkv_w[:, 0:2048].astype(bf)
    con["wk"] = qkv_w[:, 2048:4096].astype(bf)
    con["wv"] = qkv_w[:, 4096:6144].astype(bf)
    con["ow"] = np.asarray(inputs["out_w"], f32).astype(bf)
    w1 = np.zeros((2048, DFFP), f32)
    w1[:, :DFF] = np.asarray(inputs["w1"], f32)
    con["w1"] = w1.astype(bf)
    w2 = np.zeros((2048, DFFP), f32)
    w2[:, :DFF] = np.asarray(inputs["w2"], f32)
    con["w2"] = w2.astype(bf)
    w3 = np.zeros((DFFP, 2048), f32)
    w3[:DFF, :] = np.asarray(inputs["w3"], f32)
    con["w3"] = w3.astype(bf)
    return con


def host_core_inputs(inputs, con, c):
    f32 = np.float32
    bf = ml_dtypes.bfloat16
    b, half = c // 2, c % 2
    qoff = half * KC
    m = dict(con)
    m["xb"] = np.ascontiguousarray(np.asarray(inputs["x"], f32)[b])
    # causal multiplicative mask on ranks: [4kc][128k, 256q]: 1 if k_rank <= qoff+q
    kr = np.arange(K)[:, None]
    qr = (qoff + np.arange(KC))[None, :]
    mask = (kr <= qr).astype(f32).reshape(4, 128, KC).transpose(1, 0, 2)
    m["cmask"] = np.ascontiguousarray(mask).astype(bf)
    m["qs0"] = np.full((128, 1), 1.0 - half, f32)
    m["qs1"] = np.full((128, 1), float(half), f32)
    return m


_BUILT = None


def _build_program():
    global _BUILT
    if _BUILT is not None:
        return _BUILT
    nc = bacc.Bacc("TRN2", target_bir_lowering=False, debug=False,
                   enable_asserts=True, num_devices=8)
    in_specs = {
        "xb": ((T, D), F32), "rw": ((128, 2048), F32),
        "tie": ((128, 32), F32), "iota1": ((128, 32), F32),
        "ones1": ((1, 128), F32), "identb": ((128, 128), BF16),
        "n1w": ((128, 2048), BF16), "n2w": ((128, 2048), BF16),
        "cmask": ((128, 4, 256), BF16),
        "qs0": ((128, 1), F32), "qs1": ((128, 1), F32),
        "wq": ((2048, 2048), BF16), "wk": ((2048, 2048), BF16),
        "wv": ((2048, 2048), BF16), "ow": ((2048, 2048), BF16),
        "w1": ((2048, DFFP), BF16), "w2": ((2048, DFFP), BF16),
        "w3": ((DFFP, 2048), BF16),
    }
    out_specs = {
        "proc": ((KC, D), F32), "idxo": ((K,), mybir.dt.int32),
        "nfo": ((1, 1), mybir.dt.uint32),
    }
    ins = {k: nc.dram_tensor(k, s, d, kind="ExternalInput").ap()
           for k, (s, d) in in_specs.items()}
    outs = {k: nc.dram_tensor(k, s, d, kind="ExternalOutput").ap()
            for k, (s, d) in out_specs.items()}
    with tile.TileContext(nc) as tc:
        build_kernel(tc, outs, ins)
    nc.compile()
    _BUILT = nc
    return nc


def kernel(**inputs):
    from concourse import bass_utils
    from concourse.bass_interp import get_hw_module
    import copy as _copy

    nc = _build_program()
    con = host_constants(inputs)
    in_maps = [host_core_inputs(inputs, con, c) for c in range(8)]

    old_m = nc.m
    nc.m = get_hw_module(nc.m)
    try:
        res = bass_utils.run_bass_kernel_spmd(
            nc, in_maps, core_ids=list(range(8)))
    finally:
        nc.m = old_m

    x = np.asarray(inputs["x"], np.float32)
    out = x.copy()
    for g in range(B):
        idx = np.asarray(res.results[2 * g]["idxo"]).astype(np.int64)
        proc0 = np.asarray(res.results[2 * g]["proc"])
        proc1 = np.asarray(res.results[2 * g + 1]["proc"])
        out[g, idx[0:KC]] = proc0
        out[g, idx[KC:K]] = proc1
    return out



# revision 1
# speedup vs baseline: 1.0465x; 1.0465x over previous
"""Trainium2 Bass kernel for nn_MoDBlock (mixture-of-depths block), 8 cores.

Contract: kernel(**inputs) takes FULL inputs (x (4,4096,2048) f32,
position_ids (4,4096) i32 [arange per spec], router_w, norm weights, qkv_w,
out_w, w1/w2/w3) and returns the FULL (4,4096,2048) f32 output.

Sharding: 4 pairs x 2 cores; pair g owns batch row b=g. Both cores of a pair
run the router (fp32 scores + tie), exact top-512 via gpsimd kth_largest ->
threshold -> sparse_gather compaction (ascending token order, matching
jax.lax.top_k + sort semantics incl. stable tie handling), and dma_gather of
the selected rows. Core half h processes selected ranks [256h, 256h+256):
q/attention-out/out-proj/SwiGLU for its ranks; k/v projections for all 512.
Causal mask on ranks == mask on original positions (positions ascending).
Block math in bf16 with fp32 accumulation; router and residuals in fp32.
Host only shards inputs, converts weights to bf16, and scatters per-core
outputs into a copy of x (out[b, idx[...]] = proc).
"""


import os
import numpy as np
import ml_dtypes
import concourse.bass as bass
import concourse.bacc as bacc
import concourse.mybir as mybir
import concourse.tile as tile
from concourse import library_config
from concourse.tile_rust import add_dep_helper

F32 = mybir.dt.float32
BF16 = mybir.dt.bfloat16
AF = mybir.ActivationFunctionType
OP = mybir.AluOpType

B, T, D, H = 4, 4096, 2048, 16
HD = 128
K = 512
KC = 256          # tokens per core
DFF = 5461
DFFP = 5504       # padded to 43*128
NFC = DFFP // 128  # 43
EPS = 1e-6
ISQ = 1.0 / np.sqrt(128.0)
QUANT = 1.0 - 510.5 / 4095.0  # k_adj = 510 -> out {lerp, desc[511]}
DEBUG = False
PHASE_LIMIT = 8


def build_kernel(tc: tile.TileContext, outs, ins):
    nc = tc.nc
    xb = ins["xb"]
    proc_o, idx_o, nf_o = outs["proc"], outs["idxo"], outs["nfo"]

    _open = {}

    def popen(name, side="left", **kw):
        cm = tc.tile_pool(name=name, side=side, **kw)
        _open[name] = cm
        return cm.__enter__()

    def pclose(name):
        _open.pop(name).__exit__(None, None, None)

    const = popen("const", bufs=1)
    small = popen("small", bufs=1)
    x1p = popen("x1_pool", bufs=1)          # lives A->H
    xown_pool = popen("xown_pool", bufs=1)  # lives A->E
    h1T_pool = popen("h1T_pool", side="right", bufs=1)  # lives A->C

    x1_t = x1p.tile([128, 2, 2048], F32, name="x1_t")
    xown_t = xown_pool.tile([128, 2, 2048], F32, name="xown_t")
    h1T = h1T_pool.tile([128, 16, 512], BF16, name="h1T")
    h1sel = h1T_pool.tile([128, 16, 256], BF16, name="h1sel")

    # ---- constants ----
    rw_t = const.tile([128, 2048], F32)
    nc.sync.dma_start(rw_t[:], ins["rw"][:])
    tie_t = const.tile([128, 32], F32)
    nc.sync.dma_start(tie_t[:], ins["tie"][:])
    iota_t = const.tile([128, 32], F32)
    nc.sync.dma_start(iota_t[:], ins["iota1"][:])
    ones1_t = const.tile([1, 128], F32)
    nc.sync.dma_start(ones1_t[:], ins["ones1"][:])
    ident_t = const.tile([128, 128], BF16)
    nc.sync.dma_start(ident_t[:], ins["identb"][:])
    n1w_t = const.tile([128, 2048], BF16)
    nc.sync.dma_start(n1w_t[:], ins["n1w"][:])
    n2w_t = const.tile([128, 2048], BF16)
    nc.sync.dma_start(n2w_t[:], ins["n2w"][:])
    cmask_t = const.tile([128, 4, 256], BF16)
    nc.sync.dma_start(cmask_t[:], ins["cmask"][:])
    qs0_t = const.tile([128, 1], F32)
    nc.sync.dma_start(qs0_t[:], ins["qs0"][:])
    qs1_t = const.tile([128, 1], F32)
    nc.sync.dma_start(qs1_t[:], ins["qs1"][:])
    onesk_t = const.tile([128, 1], BF16)
    nc.vector.memset(onesk_t[:], 1.0)
    eps_t = const.tile([128, 1], F32)
    nc.vector.memset(eps_t[:], EPS)

    # =========== Phase A: router scores + topk + gather ===========
    S_t = small.tile([128, 32], F32)
    with tc.tile_pool(name="xstream", side="right", bufs=4) as xs:
        for k in range(32):
            xk = xs.tile([128, 2048], F32, tag="xk", name=f"xk{k}")
            nc.sync.dma_start(xk[:], xb[k * 128:(k + 1) * 128, :])
            nc.vector.scalar_tensor_tensor(
                out=xk[:], in0=xk[:], scalar=1.0, in1=rw_t[:],
                op0=OP.mult, op1=OP.mult, accum_out=S_t[:, k:k + 1],
            )
    nc.vector.tensor_add(out=S_t[:], in0=S_t[:], in1=tie_t[:])

    kth_t = small.tile([1, 2], F32)
    lib_attn = nc.gpsimd.load_library(library_config.attn)
    kth = nc.gpsimd.kth_largest(
        kth_t[:], S_t[:], n_per_lane=32, k=510, quantile=QUANT)
    add_dep_helper(kth.ins, lib_attn.ins, reason="lib attn first")

    th_t = small.tile([128, 1], F32)
    with tc.tile_pool(name="psA", bufs=1, space="PSUM") as psA:
        th_ps = psA.tile([128, 1], F32)
        nc.tensor.matmul(th_ps[:], ones1_t[:], kth_t[:, 1:2],
                         start=True, stop=True)
        nc.vector.tensor_copy(th_t[:], th_ps[:])

    cand_t = small.tile([128, 32], F32)
    nc.vector.scalar_tensor_tensor(
        out=cand_t[:], in0=S_t[:], scalar=th_t[:], in1=iota_t[:],
        op0=OP.is_ge, op1=OP.mult)
    nc.vector.tensor_scalar_add(cand_t[:], cand_t[:], -1.0)

    c16_t = small.tile([16, 32, 8], F32)
    for pi in range(8):
        nc.sync.dma_start(c16_t[:, :, pi], cand_t[pi * 16:(pi + 1) * 16, :])

    sg_t = small.tile([16, 33], F32)
    nf_t = small.tile([1, 1], mybir.dt.uint32)
    lib_sg = nc.gpsimd.load_library(library_config.sparse_gather)
    sg = nc.gpsimd.sparse_gather(
        sg_t[:], c16_t[:].rearrange("p k j -> p (k j)"), num_found=nf_t[:])
    add_dep_helper(lib_sg.ins, kth.ins, reason="lib switch after kth")
    add_dep_helper(sg.ins, lib_sg.ins, reason="sg after lib")
    nc.sync.dma_start(nf_o[:], nf_t[:])

    idx32_t = small.tile([16, 32], mybir.dt.int32)
    nc.vector.tensor_copy(idx32_t[:], sg_t[:, 0:32])
    nc.sync.dma_start(idx_o.rearrange("(f p) -> p f", p=16), idx32_t[:])

    idx16_t = small.tile([16, 32], mybir.dt.int16)
    nc.vector.tensor_copy(idx16_t[:], sg_t[:, 0:32])
    idx128_t = small.tile([128, 32], mybir.dt.int16)
    for g in range(8):
        nc.sync.dma_start(idx128_t[g * 16:(g + 1) * 16, :], idx16_t[:])

    x_sel = popen("x_sel_pool", bufs=1)
    xsel_t = x_sel.tile([128, 4, 2048], F32, name="xsel_t")
    lib_mlp = nc.gpsimd.load_library(library_config.mlp)
    gat = nc.gpsimd.dma_gather(
        xsel_t[:], xb[:], idx128_t[:], K, K, 2048)
    add_dep_helper(lib_mlp.ins, sg.ins, reason="lib switch after sg")
    add_dep_helper(gat.ins, lib_mlp.ins, reason="gather after lib")


    if PHASE_LIMIT <= 1:
        nc.sync.dma_start(proc_o[0:128, :], x1_t[:, 0, :] if 1 >= 5 else xown_t[:, 0, :])
        for nm in list(_open)[::-1]:
            pclose(nm)
        return
    # =========== Phase B: norm1, h1, h1T, blends ===========
    rs1_t = small.tile([128, 4], F32)
    sq1_t = small.tile([128, 4], F32)
    with tc.tile_pool(name="scratch", bufs=2) as scr:
        for c in range(4):
            sc = scr.tile([128, 2048], F32, tag="sc", name=f"sc{c}")
            nc.vector.scalar_tensor_tensor(
                out=sc[:], in0=xsel_t[:, c, :], scalar=1.0,
                in1=xsel_t[:, c, :], op0=OP.mult, op1=OP.mult,
                accum_out=sq1_t[:, c:c + 1])
    nc.scalar.activation(rs1_t[:], sq1_t[:], AF.Sqrt,
                         scale=1.0 / 2048.0, bias=eps_t[:])
    nc.vector.reciprocal(rs1_t[:], rs1_t[:])

    with (
        tc.tile_pool(name="h1_pool", bufs=1) as h1p,
        tc.tile_pool(name="psT", bufs=4, space="PSUM") as psT,
    ):
        h1_t = h1p.tile([128, 4, 2048], BF16, name="h1_t")
        for c in range(4):
            nc.vector.scalar_tensor_tensor(
                out=h1_t[:, c, :], in0=xsel_t[:, c, :],
                scalar=rs1_t[:, c:c + 1], in1=n1w_t[:],
                op0=OP.mult, op1=OP.mult)
        for c in range(4):
            for dc in range(16):
                pt = psT.tile([128, 128], BF16, tag="pt", name=f"pt{c}_{dc}")
                nc.tensor.transpose(
                    pt[:], h1_t[:, c, dc * 128:(dc + 1) * 128], ident_t[:])
                nc.scalar.activation(
                    h1T[:, dc, c * 128:(c + 1) * 128], pt[:], AF.Copy)

    # x_own / h1sel blends
    for qt in range(2):
        nc.vector.tensor_scalar_mul(
            xown_t[:, qt, :], xsel_t[:, qt, :], qs0_t[:])
        nc.vector.scalar_tensor_tensor(
            out=xown_t[:, qt, :], in0=xsel_t[:, 2 + qt, :],
            scalar=qs1_t[:], in1=xown_t[:, qt, :],
            op0=OP.mult, op1=OP.add)
    for dc in range(16):
        nc.vector.tensor_scalar_mul(
            h1sel[:, dc, :], h1T[:, dc, 0:256], qs0_t[:])
        nc.vector.scalar_tensor_tensor(
            out=h1sel[:, dc, :], in0=h1T[:, dc, 256:512],
            scalar=qs1_t[:], in1=h1sel[:, dc, :],
            op0=OP.mult, op1=OP.add)
    pclose("x_sel_pool")


    if PHASE_LIMIT <= 2:
        nc.sync.dma_start(proc_o[0:128, :], x1_t[:, 0, :] if 2 >= 5 else xown_t[:, 0, :])
        for nm in list(_open)[::-1]:
            pclose(nm)
        return
    # =========== Phase C: qkv projections ===========
    qkvp = popen("qkv_pool", bufs=1)
    qT = qkvp.tile([128, 16, 256], BF16, name="qT")
    kT = qkvp.tile([128, 16, 512], BF16, name="kT")
    V = qkvp.tile([128, 4, 2048], BF16, name="V")

    with (
        tc.tile_pool(name="wqk_stream", bufs=3) as wqs,
        tc.tile_pool(name="psC", bufs=2, space="PSUM") as psC,
    ):
        for jc in range(16):
            wqc = wqs.tile([128, 16, 128], BF16, tag="wqc", name=f"wq{jc}")
            nc.sync.dma_start(
                wqc[:], ins["wq"].rearrange("(dc p) j -> p dc j", p=128)
                [:, :, jc * 128:(jc + 1) * 128])
            pq = psC.tile([128, 256], F32, tag="pq", bufs=2, name=f"pq{jc}")
            for dc in range(16):
                nc.tensor.matmul(pq[:], wqc[:, dc, :], h1sel[:, dc, :],
                                 start=(dc == 0), stop=(dc == 15))
            nc.scalar.activation(qT[:, jc, :], pq[:], AF.Copy)
        for jc in range(16):
            wkc = wqs.tile([128, 16, 128], BF16, tag="wqc", name=f"wk{jc}")
            nc.sync.dma_start(
                wkc[:], ins["wk"].rearrange("(dc p) j -> p dc j", p=128)
                [:, :, jc * 128:(jc + 1) * 128])
            pk = psC.tile([128, 512], F32, tag="pk", bufs=2, name=f"pk{jc}")
            for dc in range(16):
                nc.tensor.matmul(pk[:], wkc[:, dc, :], h1T[:, dc, :],
                                 start=(dc == 0), stop=(dc == 15))
            nc.scalar.activation(kT[:, jc, :], pk[:], AF.Copy)
        for vc in range(4):
            pvs = [psC.tile([128, 512], F32, tag="pv", bufs=4,
                            name=f"pv{vc}_{i}") for i in range(4)]
            for dc in range(16):
                wvc = wqs.tile([128, 512], BF16, tag="wvc", name=f"wv{vc}_{dc}")
                nc.sync.dma_start(
                    wvc[:], ins["wv"][dc * 128:(dc + 1) * 128,
                                      vc * 512:(vc + 1) * 512])
                for tc4 in range(4):
                    nc.tensor.matmul(
                        pvs[tc4][:], h1T[:, dc, tc4 * 128:(tc4 + 1) * 128],
                        wvc[:], start=(dc == 0), stop=(dc == 15))
            for tc4 in range(4):
                nc.scalar.activation(
                    V[:, tc4, vc * 512:(vc + 1) * 512], pvs[tc4][:], AF.Copy)
    pclose("h1T_pool")


    if PHASE_LIMIT <= 3:
        nc.sync.dma_start(proc_o[0:128, :], x1_t[:, 0, :] if 3 >= 5 else xown_t[:, 0, :])
        for nm in list(_open)[::-1]:
            pclose(nm)
        return
    # =========== Phase D: attention ===========
    attp = popen("att_pool", side="right", bufs=1)
    o_t = attp.tile([128, 2, 16, 128], BF16, name="o_t")
    oT = attp.tile([128, 16, 256], BF16, name="oT")
    with (
        tc.tile_pool(name="pT_pool", bufs=2) as pTp,
        tc.tile_pool(name="lrow_pool", bufs=2) as lrp,
        tc.tile_pool(name="psD", bufs=2, space="PSUM") as psD,
        tc.tile_pool(name="psL", bufs=2, space="PSUM") as psL,
    ):
        for h in range(16):
            pT = pTp.tile([128, 4, 256], BF16, tag="pT", name=f"pT{h}")
            for kc in range(4):
                ss = psD.tile([128, 256], F32, tag="ss", name=f"ss{h}_{kc}")
                nc.tensor.matmul(
                    ss[:], kT[:, h, kc * 128:(kc + 1) * 128], qT[:, h, :],
                    start=True, stop=True)
                pe_t = pTp.tile([128, 256], F32, tag="pe", name=f"pe{h}_{kc}")
                nc.scalar.activation(pe_t[:], ss[:], AF.Exp, scale=ISQ)
                nc.vector.tensor_mul(
                    out=pT[:, kc, :], in0=pe_t[:], in1=cmask_t[:, kc, :])
            lps = psL.tile([1, 256], F32, tag="lps", name=f"lps{h}")
            for kc in range(4):
                nc.tensor.matmul(lps[:], onesk_t[:], pT[:, kc, :],
                                 start=(kc == 0), stop=(kc == 3))
            lrow = lrp.tile([1, 256], F32, tag="lrow", name=f"lrow{h}")
            nc.scalar.activation(lrow[:], lps[:], AF.Copy)
            lcol = lrp.tile([128, 2], F32, tag="lcol", name=f"lcol{h}")
            for qt in range(2):
                nc.sync.dma_start(
                    lcol[:, qt:qt + 1],
                    lrow[0:1, qt * 128:(qt + 1) * 128])
            rL = lrp.tile([128, 2], F32, tag="rL", name=f"rL{h}")
            nc.vector.reciprocal(rL[:], lcol[:])
            if DEBUG and h == 0:
                nc.sync.dma_start(outs["dbg_pT"][:], pT[:])
                nc.sync.dma_start(outs["dbg_rL"][:], rL[:])
                nc.sync.dma_start(outs["dbg_lrow"][:], lrow[:])
            for qt in range(2):
                po = psD.tile([128, 128], F32, tag="po", name=f"po{h}_{qt}")
                for kc in range(4):
                    nc.tensor.matmul(
                        po[:], pT[:, kc, qt * 128:(qt + 1) * 128],
                        V[:, kc, h * 128:(h + 1) * 128],
                        start=(kc == 0), stop=(kc == 3))
                nc.scalar.activation(o_t[:, qt, h, :], po[:], AF.Copy,
                                     scale=rL[:, qt:qt + 1])
    with tc.tile_pool(name="psT2", bufs=2, space="PSUM") as psT2:
        for qt in range(2):
            for h in range(16):
                pt = psT2.tile([128, 128], BF16, tag="pt2",
                               name=f"pt2_{qt}_{h}")
                nc.tensor.transpose(pt[:], o_t[:, qt, h, :], ident_t[:])
                nc.scalar.activation(
                    oT[:, h, qt * 128:(qt + 1) * 128], pt[:], AF.Copy)
    if DEBUG:
        nc.sync.dma_start(outs["dbg_qT"][:], qT[:])
        nc.sync.dma_start(outs["dbg_kT"][:], kT[:])
        nc.sync.dma_start(outs["dbg_V"][:], V[:])
        nc.sync.dma_start(outs["dbg_o"][:], o_t[:])
    pclose("qkv_pool")


    if PHASE_LIMIT <= 4:
        nc.sync.dma_start(proc_o[0:128, :], x1_t[:, 0, :] if 4 >= 5 else xown_t[:, 0, :])
        for nm in list(_open)[::-1]:
            pclose(nm)
        return
    # =========== Phase E: out-proj + residual -> x1 ===========
    with (
        tc.tile_pool(name="ow_stream", bufs=2) as ows,
        tc.tile_pool(name="psE", bufs=2, space="PSUM") as psE,
    ):
        for nc4 in range(4):
            owc = ows.tile([128, 16, 512], BF16, tag="owc", name=f"ow{nc4}")
            nc.sync.dma_start(
                owc[:], ins["ow"].rearrange("(oc p) n -> p oc n", p=128)
                [:, :, nc4 * 512:(nc4 + 1) * 512])
            for qt in range(2):
                poo = psE.tile([128, 512], F32, tag="poo",
                               name=f"poo{nc4}_{qt}")
                for oc in range(16):
                    nc.tensor.matmul(
                        poo[:], oT[:, oc, qt * 128:(qt + 1) * 128],
                        owc[:, oc, :], start=(oc == 0), stop=(oc == 15))
                nc.vector.tensor_add(
                    out=x1_t[:, qt, nc4 * 512:(nc4 + 1) * 512],
                    in0=poo[:], in1=xown_t[:, qt, nc4 * 512:(nc4 + 1) * 512])
    if DEBUG:
        nc.sync.dma_start(outs["dbg_x1"][:], x1_t[:])
        nc.sync.dma_start(outs["dbg_xown"][:], xown_t[:])
    pclose("att_pool")
    pclose("xown_pool")


    if PHASE_LIMIT <= 5:
        nc.sync.dma_start(proc_o[0:128, :], x1_t[:, 0, :] if 5 >= 5 else xown_t[:, 0, :])
        for nm in list(_open)[::-1]:
            pclose(nm)
        return
    # =========== Phase F: norm2 + h2T ===========
    rs2_t = small.tile([128, 2], F32)
    sq2_t = small.tile([128, 2], F32)
    zzp = popen("zz_pool", side="right", bufs=1)
    zz = zzp.tile([128, NFC, 256], BF16, name="zz")
    h2Tp = popen("h2T_pool", side="right", bufs=1)
    h2T = h2Tp.tile([128, 16, 256], BF16, name="h2T")

    with tc.tile_pool(name="scratch2", bufs=2) as scr2:
        for c in range(2):
            sc = scr2.tile([128, 2048], F32, tag="sc2", name=f"sc2_{c}")
            nc.vector.scalar_tensor_tensor(
                out=sc[:], in0=x1_t[:, c, :], scalar=1.0,
                in1=x1_t[:, c, :], op0=OP.mult, op1=OP.mult,
                accum_out=sq2_t[:, c:c + 1])
    nc.scalar.activation(rs2_t[:], sq2_t[:], AF.Sqrt,
                         scale=1.0 / 2048.0, bias=eps_t[:])
    nc.vector.reciprocal(rs2_t[:], rs2_t[:])

    with (
        tc.tile_pool(name="h2_pool", bufs=1) as h2p,
        tc.tile_pool(name="psT3", bufs=2, space="PSUM") as psT3,
    ):
        h2_t = h2p.tile([128, 2, 2048], BF16, name="h2_t")
        for c in range(2):
            nc.vector.scalar_tensor_tensor(
                out=h2_t[:, c, :], in0=x1_t[:, c, :],
                scalar=rs2_t[:, c:c + 1], in1=n2w_t[:],
                op0=OP.mult, op1=OP.mult)
        for c in range(2):
            for dc in range(16):
                pt = psT3.tile([128, 128], BF16, tag="pt3",
                               name=f"pt3_{c}_{dc}")
                nc.tensor.transpose(
                    pt[:], h2_t[:, c, dc * 128:(dc + 1) * 128], ident_t[:])
                nc.scalar.activation(
                    h2T[:, dc, c * 128:(c + 1) * 128], pt[:], AF.Copy)


    if PHASE_LIMIT <= 6:
        nc.sync.dma_start(proc_o[0:128, :], x1_t[:, 0, :] if 6 >= 5 else xown_t[:, 0, :])
        for nm in list(_open)[::-1]:
            pclose(nm)
        return
    # =========== Phase G: FFN ===========
    with (
        tc.tile_pool(name="w12_stream", bufs=3) as w12s,
        tc.tile_pool(name="sig_pool", bufs=3) as sigp,
        tc.tile_pool(name="psG", bufs=2, space="PSUM") as psG,
    ):
        for fc in range(NFC):
            w1c = w12s.tile([128, 16, 128], BF16, tag="w1c", name=f"w1c{fc}")
            nc.sync.dma_start(
                w1c[:], ins["w1"].rearrange("(dc p) f -> p dc f", p=128)
                [:, :, fc * 128:(fc + 1) * 128])
            w2c = w12s.tile([128, 16, 128], BF16, tag="w2c", name=f"w2c{fc}")
            nc.sync.dma_start(
                w2c[:], ins["w2"].rearrange("(dc p) f -> p dc f", p=128)
                [:, :, fc * 128:(fc + 1) * 128])
            p1 = psG.tile([128, 256], F32, tag="p1", name=f"p1_{fc}")
            p2 = psG.tile([128, 256], F32, tag="p2", name=f"p2_{fc}")
            for dc in range(16):
                nc.tensor.matmul(p1[:], w1c[:, dc, :], h2T[:, dc, :],
                                 start=(dc == 0), stop=(dc == 15))
            for dc in range(16):
                nc.tensor.matmul(p2[:], w2c[:, dc, :], h2T[:, dc, :],
                                 start=(dc == 0), stop=(dc == 15))
            sg2 = sigp.tile([128, 256], BF16, tag="sg2", name=f"sg2_{fc}")
            nc.scalar.activation(sg2[:], p1[:], AF.Sigmoid)
            s1 = sigp.tile([128, 256], BF16, tag="s1", name=f"s1_{fc}")
            nc.vector.tensor_mul(out=s1[:], in0=sg2[:], in1=p1[:])
            nc.vector.tensor_mul(out=zz[:, fc, :], in0=s1[:], in1=p2[:])
    pclose("h2T_pool")


    if PHASE_LIMIT <= 7:
        nc.sync.dma_start(proc_o[0:128, :], x1_t[:, 0, :] if 7 >= 5 else xown_t[:, 0, :])
        for nm in list(_open)[::-1]:
            pclose(nm)
        return
    # =========== Phase H: w3 + residual -> proc ===========
    procp = popen("proc_pool", bufs=1)
    proc_t = procp.tile([128, 2, 2048], F32, name="proc_t")
    with (
        tc.tile_pool(name="w3_stream", bufs=4) as w3s,
        tc.tile_pool(name="psH", bufs=1, space="PSUM") as psH,
    ):
        pffs = {}
        for qt in range(2):
            for nc4 in range(4):
                pffs[(qt, nc4)] = psH.tile(
                    [128, 512], F32, tag=f"pff{qt}{nc4}",
                    name=f"pff{qt}{nc4}")
        for fc in range(NFC):
            w3c = w3s.tile([128, 2048], BF16, tag="w3c", name=f"w3c{fc}")
            nc.sync.dma_start(w3c[:], ins["w3"][fc * 128:(fc + 1) * 128, :])
            for qt in range(2):
                for nc4 in range(4):
                    nc.tensor.matmul(
                        pffs[(qt, nc4)][:],
                        zz[:, fc, qt * 128:(qt + 1) * 128],
                        w3c[:, nc4 * 512:(nc4 + 1) * 512],
                        start=(fc == 0), stop=(fc == NFC - 1))
        for qt in range(2):
            for nc4 in range(4):
                nc.vector.tensor_add(
                    out=proc_t[:, qt, nc4 * 512:(nc4 + 1) * 512],
                    in0=pffs[(qt, nc4)][:],
                    in1=x1_t[:, qt, nc4 * 512:(nc4 + 1) * 512])
    for qt in range(2):
        nc.sync.dma_start(proc_o[qt * 128:(qt + 1) * 128, :],
                          proc_t[:, qt, :])
    pclose("proc_pool")
    pclose("zz_pool")
    pclose("x1_pool")
    pclose("xown_pool") if "xown_pool" in _open else None
    pclose("small")
    pclose("const")


# ======================= host side =======================

def host_constants(inputs):
    """Shared per-core constants from full inputs (numpy)."""
    f32 = np.float32
    bf = ml_dtypes.bfloat16
    qkv_w = np.asarray(inputs["qkv_w"], f32)
    con = {}
    con["rw"] = np.broadcast_to(
        np.asarray(inputs["router_w"], f32)[None, :], (128, 2048)).copy()
    tie = (np.arange(T, dtype=f32) * np.float32(1e-6))
    con["tie"] = tie.reshape(32, 128).T.copy()
    con["iota1"] = (np.arange(T, dtype=f32) + 1.0).reshape(32, 128).T.copy().astype(f32)
    con["ones1"] = np.ones((1, 128), f32)
    con["identb"] = np.eye(128, dtype=f32).astype(bf)
    con["n1w"] = np.broadcast_to(
        np.asarray(inputs["norm1_w"], f32)[None, :], (128, 2048)).astype(bf)
    con["n2w"] = np.broadcast_to(
        np.asarray(inputs["norm2_w"], f32)[None, :], (128, 2048)).astype(bf)
    con["wq"] = qkv_w[:, 0:2048].astype(bf)
    con["wk"] = qkv_w[:, 2048:4096].astype(bf)
    con["wv"] = qkv_w[:, 4096:6144].astype(bf)
    con["ow"] = np.asarray(inputs["out_w"], f32).astype(bf)
    w1 = np.zeros((2048, DFFP), f32)
    w1[:, :DFF] = np.asarray(inputs["w1"], f32)
    con["w1"] = w1.astype(bf)
    w2 = np.zeros((2048, DFFP), f32)
    w2[:, :DFF] = np.asarray(inputs["w2"], f32)
    con["w2"] = w2.astype(bf)
    w3 = np.zeros((DFFP, 2048), f32)
    w3[:DFF, :] = np.asarray(inputs["w3"], f32)
    con["w3"] = w3.astype(bf)
    return con


def host_core_inputs(inputs, con, c):
    f32 = np.float32
    bf = ml_dtypes.bfloat16
    b, half = c // 2, c % 2
    qoff = half * KC
    m = dict(con)
    m["xb"] = np.ascontiguousarray(np.asarray(inputs["x"], f32)[b])
    # causal multiplicative mask on ranks: [4kc][128k, 256q]: 1 if k_rank <= qoff+q
    kr = np.arange(K)[:, None]
    qr = (qoff + np.arange(KC))[None, :]
    mask = (kr <= qr).astype(f32).reshape(4, 128, KC).transpose(1, 0, 2)
    m["cmask"] = np.ascontiguousarray(mask).astype(bf)
    m["qs0"] = np.full((128, 1), 1.0 - half, f32)
    m["qs1"] = np.full((128, 1), float(half), f32)
    return m


_BUILT = None


def _build_program():
    global _BUILT
    if _BUILT is not None:
        return _BUILT
    nc = bacc.Bacc("TRN2", target_bir_lowering=False, debug=False,
                   enable_asserts=True, num_devices=8)
    in_specs = {
        "xb": ((T, D), F32), "rw": ((128, 2048), F32),
        "tie": ((128, 32), F32), "iota1": ((128, 32), F32),
        "ones1": ((1, 128), F32), "identb": ((128, 128), BF16),
        "n1w": ((128, 2048), BF16), "n2w": ((128, 2048), BF16),
        "cmask": ((128, 4, 256), BF16),
        "qs0": ((128, 1), F32), "qs1": ((128, 1), F32),
        "wq": ((2048, 2048), BF16), "wk": ((2048, 2048), BF16),
        "wv": ((2048, 2048), BF16), "ow": ((2048, 2048), BF16),
        "w1": ((2048, DFFP), BF16), "w2": ((2048, DFFP), BF16),
        "w3": ((DFFP, 2048), BF16),
    }
    out_specs = {
        "proc": ((KC, D), F32), "idxo": ((K,), mybir.dt.int32),
        "nfo": ((1, 1), mybir.dt.uint32),
    }
    ins = {k: nc.dram_tensor(k, s, d, kind="ExternalInput").ap()
           for k, (s, d) in in_specs.items()}
    outs = {k: nc.dram_tensor(k, s, d, kind="ExternalOutput").ap()
            for k, (s, d) in out_specs.items()}
    with tile.TileContext(nc) as tc:
        build_kernel(tc, outs, ins)
    nc.compile()
    _BUILT = nc
    return nc


def kernel(**inputs):
    from concourse import bass_utils
    from concourse.bass_interp import get_hw_module
    import copy as _copy

    nc = _build_program()
    con = host_constants(inputs)
    in_maps = [host_core_inputs(inputs, con, c) for c in range(8)]

    old_m = nc.m
    nc.m = get_hw_module(nc.m)
    try:
        res = bass_utils.run_bass_kernel_spmd(
            nc, in_maps, core_ids=list(range(8)))
    finally:
        nc.m = old_m

    x = np.asarray(inputs["x"], np.float32)
    out = x.copy()
    for g in range(B):
        idx = np.asarray(res.results[2 * g]["idxo"]).astype(np.int64)
        proc0 = np.asarray(res.results[2 * g]["proc"])
        proc1 = np.asarray(res.results[2 * g + 1]["proc"])
        out[g, idx[0:KC]] = proc0
        out[g, idx[KC:K]] = proc1
    return out

